# revision 1
# baseline (speedup 1.0000x reference)
"""Grouped multi-query attention (group axis summed) on 8 trn2 NeuronCores.

Math: reference sums the g axis of the grouped Q heads inside the score
einsum, so the whole module collapses to standard 8-head attention with
W_Qeff[n] = sum_g W_Q[4n+g] (and the 1/sqrt(64) score scale folded in).

Sharding: core c -> (batch b = c//2, kv-head half = c%2). Each core runs
4 heads of one batch and produces a full [2048, 2048] partial of the
output projection; the host sums the two halves per batch.

Per-core layout (all matmuls are out = lhsT.T @ rhs, bf16 in / f32 acc):
  xT [d, t] host-pretransposed; Q_T/K_T [2*64 head-pair rows, t] built by
  projection matmuls; V [t, 4*64] built directly; scores computed
  transposed S_T[k, q] = K @ Q^T so softmax/PV need no transposes at all;
  per-head Z lands in psum partitions 64*rj..64*rj+63 via tile_position
  col offsets; softmax denominators come from a packed M=1 ones-matmul,
  extracted across partitions with a tiny DMA, inverted, and broadcast
  back over 64 partitions with a K=1 ones-matmul. The two heads of a pair
  run concurrently in disjoint PE row/col groups (tile_position packing);
  causal masking is a multiplicative bf16 {0,1} DVE mult on the exp'd
  tile (4x mode); out-projection is interleaved per q-block round.
"""

import numpy as np

S = 2048
D = 2048
HD = 64
NKV = 8
GQ = 4  # grouped q heads per kv head (summed)
HPC = 4  # heads per core
TB = 512
QB = 512
NTB = S // TB
NDC = D // 128
NQB = S // QB
NKT = S // 128
IGNORE = -100000.0

_CACHE = {}


def _build_program():
    import concourse.bass as bass
    import concourse.tile as tile
    from concourse import bacc, mybir

    F32 = mybir.dt.float32
    BF16 = mybir.dt.bfloat16
    AF = mybir.ActivationFunctionType

    nc = bacc.Bacc("TRN2", target_bir_lowering=False, debug=False,
                   enable_asserts=False, num_devices=8)

    xT_d = nc.dram_tensor("xT", [D, S], BF16, kind="ExternalInput").ap()
    wq_d = nc.dram_tensor("wq", [D, HPC * HD], BF16, kind="ExternalInput").ap()
    wk_d = nc.dram_tensor("wk", [D, HPC * HD], BF16, kind="ExternalInput").ap()
    wv_d = nc.dram_tensor("wv", [D, HPC * HD], BF16, kind="ExternalInput").ap()
    wo_d = nc.dram_tensor("wo", [HPC * HD, D], BF16, kind="ExternalInput").ap()
    mask_d = nc.dram_tensor("mask", [128, 4, QB], BF16, kind="ExternalInput").ap()
    o_d = nc.dram_tensor("o", [S, D], F32, kind="ExternalOutput").ap()

    def r(ap):
        return ap

    import contextlib
    with tile.TileContext(nc) as tc, \
            nc.allow_low_precision(reason="bf16 matmul operands by design"):
        with (
            tc.tile_pool(name="singles", bufs=1) as singles,
            tc.tile_pool(name="persist", bufs=1) as persist,
            tc.tile_pool(name="work", bufs=4) as work,
            tc.tile_pool(name="outsb", bufs=3) as outsb,
            tc.tile_pool(name="tiny", bufs=4) as tiny,
            tc.tile_pool(name="bcsb", bufs=2) as bcsb,
        ):
            # constants + weights
            wq_sb = singles.tile([128, NDC, HPC * HD], BF16, tag="wq")
            wk_sb = singles.tile([128, NDC, HPC * HD], BF16, tag="wk")
            wv_sb = singles.tile([128, NDC, HPC * HD], BF16, tag="wv")
            wo_sb = singles.tile([128, 2, D], BF16, tag="wo")
            mask_sb = singles.tile([128, 4, QB], BF16, tag="mask")
            ones_col = singles.tile([128, 1], BF16, tag="onec")
            ones_row = singles.tile([1, HD], BF16, tag="oner")

            nc.sync.dma_start(out=wq_sb, in_=wq_d.rearrange("(c p) n -> p c n", p=128))
            nc.sync.dma_start(out=wk_sb, in_=wk_d.rearrange("(c p) n -> p c n", p=128))
            nc.sync.dma_start(out=wv_sb, in_=wv_d.rearrange("(c p) n -> p c n", p=128))
            nc.sync.dma_start(out=wo_sb, in_=wo_d.rearrange("(g p) d -> p g d", p=128))
            nc.sync.dma_start(out=mask_sb, in_=mask_d)
            nc.vector.memset(ones_col, 1.0)
            nc.vector.memset(ones_row, 1.0)

            qT_sb = persist.tile([128, 2, S], BF16, tag="qT")
            kT_sb = persist.tile([128, 2, S], BF16, tag="kT")
            v_sb = persist.tile([128, NKT, HPC * HD], BF16, tag="v")
            z_sb = persist.tile([128, 2, S], BF16, tag="z")

            # ---- phase 1: projections ----
            with tc.tile_pool(name="ph1ps", bufs=4,
                              space=bass.MemorySpace.PSUM) as ph1ps:
                for tb in range(NTB):
                    ps_q = [ph1ps.tile([128, TB], F32, tag="qk", name=f"psq{tb}_{i}") for i in range(2)]
                    ps_k = [ph1ps.tile([128, TB], F32, tag="qk", name=f"psk{tb}_{i}") for i in range(2)]
                    ps_v = [ph1ps.tile([128, HPC * HD], F32, tag="v", name=f"psv{tb}_{i}") for i in range(4)]
                    for dc in range(NDC):
                        xt = work.tile([128, TB], BF16, tag="xt")
                        nc.sync.dma_start(
                            out=xt,
                            in_=xT_d[dc * 128:(dc + 1) * 128, tb * TB:(tb + 1) * TB])
                        st = dict(start=(dc == 0), stop=(dc == NDC - 1))
                        for g in range(2):
                            nc.tensor.matmul(
                                ps_q[g], r(wq_sb[:, dc, 128 * g:128 * (g + 1)]),
                                r(xt), **st)
                            nc.tensor.matmul(
                                ps_k[g], r(wk_sb[:, dc, 128 * g:128 * (g + 1)]),
                                r(xt), **st)
                        for tt in range(4):
                            nc.tensor.matmul(
                                ps_v[tt], r(xt[:, tt * 128:(tt + 1) * 128]),
                                r(wv_sb[:, dc, :]), **st)
                    for g in range(2):
                        nc.scalar.copy(out=qT_sb[:, g, tb * TB:(tb + 1) * TB], in_=ps_q[g])
                        nc.scalar.copy(out=kT_sb[:, g, tb * TB:(tb + 1) * TB], in_=ps_k[g])
                    for tt in range(4):
                        nc.vector.tensor_copy(
                            out=v_sb[:, 4 * tb + tt, :], in_=ps_v[tt])

            # ---- phase 2: attention, phase 3: output projection ----
            with (
                tc.tile_pool(name="sps", bufs=2, space=bass.MemorySpace.PSUM) as sps,
                tc.tile_pool(name="zps", bufs=2, space=bass.MemorySpace.PSUM) as zps,
                tc.tile_pool(name="smps", bufs=1, space=bass.MemorySpace.PSUM) as smps,
                tc.tile_pool(name="bcps", bufs=1, space=bass.MemorySpace.PSUM) as bcps,
                tc.tile_pool(name="ops", bufs=2, space=bass.MemorySpace.PSUM) as ops,
            ):
                for jq in range(NQB):
                    nkt = 4 * (jq + 1)
                    for g in range(2):
                        # both heads of pair g packed into disjoint array
                        # regions: jj=0 -> z rows 64:128, sums row 32;
                        # jj=1 -> z rows 0:64, sums row 64
                        ps_z = zps.tile([128, QB], F32, tag="z",
                                        name=f"z{jq}_{g}")
                        ps_sm = smps.tile([128, QB], F32, tag="sm",
                                          name=f"sm{jq}_{g}")
                        for ik in range(nkt):
                            p2 = []
                            for jj in range(2):
                                ps_s = sps.tile([128, QB], F32, tag="s",
                                                name=f"s{jq}_{g}_{ik}_{jj}")
                                nc.tensor.matmul(
                                    ps_s,
                                    kT_sb[64 * jj:64 * (jj + 1), g,
                                          ik * 128:(ik + 1) * 128],
                                    qT_sb[64 * jj:64 * (jj + 1), g,
                                          jq * QB:(jq + 1) * QB],
                                    start=True, stop=True)
                                p_sb = work.tile([128, QB], BF16, tag="p",
                                                 name=f"p{jq}_{g}_{ik}_{jj}")
                                nc.scalar.activation(out=p_sb, in_=ps_s,
                                                     func=AF.Exp)
                                if ik >= 4 * jq:
                                    nc.vector.tensor_mul(
                                        p_sb, p_sb, mask_sb[:, ik - 4 * jq, :])
                                p2.append(p_sb)
                            st = dict(start=(ik == 0), stop=(ik == nkt - 1),
                                      skip_group_check=True)
                            for jj, p_sb in enumerate(p2):
                                rj = 1 - jj
                                nc.tensor.matmul(
                                    ps_z[64 * rj:64 * (rj + 1), :],
                                    v_sb[:, ik, HD * (2 * g + jj):
                                         HD * (2 * g + jj + 1)],
                                    p_sb, tile_position=(0, 64 * rj), **st)
                            for jj, p_sb in enumerate(p2):
                                sc = 32 if jj == 0 else 64
                                nc.tensor.matmul(
                                    ps_sm[sc:sc + 1, :], ones_col, p_sb,
                                    tile_position=(0, sc), **st)
                        for jj in range(2):
                            rj = 1 - jj
                            sc = 32 if jj == 0 else 64
                            sumhi = tiny.tile([65, QB], F32, tag="sumhi",
                                              name=f"sh{jq}_{g}_{jj}")
                            nc.scalar.copy(out=sumhi[sc:sc + 1, :],
                                           in_=ps_sm[sc:sc + 1, :])
                            sums_sb = tiny.tile([1, QB], F32, tag="sums",
                                                name=f"su{jq}_{g}_{jj}")
                            nc.gpsimd.dma_start(out=sums_sb,
                                                in_=sumhi[sc:sc + 1, :])
                            recip_sb = tiny.tile([1, QB], BF16, tag="recip",
                                                 name=f"re{jq}_{g}_{jj}")
                            nc.vector.reciprocal(out=recip_sb, in_=sums_sb)
                            ps_bc = bcps.tile([128, QB], F32, tag="bc",
                                              name=f"bc{jq}_{g}_{jj}")
                            nc.tensor.matmul(
                                ps_bc[64 * rj:64 * (rj + 1), :], ones_row,
                                recip_sb, tile_position=(0, 64 * rj),
                                start=True, stop=True)
                            bc_sb = bcsb.tile([128, QB], F32, tag="bc",
                                              name=f"bs{jq}_{g}_{jj}")
                            nc.scalar.copy(
                                out=bc_sb[64 * rj:64 * (rj + 1), :],
                                in_=ps_bc[64 * rj:64 * (rj + 1), :])
                            nc.vector.tensor_mul(
                                z_sb[64 * rj:64 * (rj + 1), g,
                                     jq * QB:(jq + 1) * QB],
                                ps_z[64 * rj:64 * (rj + 1), :],
                                bc_sb[64 * rj:64 * (rj + 1), :])

                    for it in range(4 * jq, 4 * jq + 4):
                        for db in range(4):
                            ps_o = ops.tile([128, 512], F32, tag="o",
                                            name=f"o{it}_{db}")
                            for g in range(2):
                                nc.tensor.matmul(
                                    ps_o, z_sb[:, g, it * 128:(it + 1) * 128],
                                    wo_sb[:, g, db * 512:(db + 1) * 512],
                                    start=(g == 0), stop=(g == 1))
                            o_sb = outsb.tile([128, 512], F32, tag="o",
                                              name=f"os{it}_{db}")
                            nc.vector.tensor_copy(out=o_sb, in_=ps_o)
                            nc.sync.dma_start(
                                out=o_d[it * 128:(it + 1) * 128,
                                        db * 512:(db + 1) * 512],
                                in_=o_sb)

    nc.compile()
    return nc


def get_program():
    if "nc" not in _CACHE:
        _CACHE["nc"] = _build_program()
    return _CACHE["nc"]


def make_in_maps(normalized_resid_pre, W_Q, W_K, W_V, W_O):
    x = normalized_resid_pre
    x = np.ascontiguousarray(np.asarray(x, np.float32))
    W_Q = np.asarray(W_Q, np.float32)
    W_K = np.asarray(W_K, np.float32)
    W_V = np.asarray(W_V, np.float32)
    W_O = np.asarray(W_O, np.float32)
    wqe = W_Q.reshape(NKV, GQ, D, HD).sum(1) * (1.0 / np.sqrt(HD))

    kk = np.arange(128)[:, None, None]
    mm = np.arange(4)[None, :, None]
    qq = np.arange(QB)[None, None, :]
    import ml_dtypes
    mask = np.where(mm * 128 + kk <= qq, 1.0, 0.0).astype(ml_dtypes.bfloat16)
    mask = np.ascontiguousarray(mask)

    in_maps = []
    for c in range(8):
        b, half = divmod(c, 2)
        heads = [4 * half + m for m in range(HPC)]
        xT = np.ascontiguousarray(x[b].T)
        wq = np.ascontiguousarray(np.concatenate([wqe[n] for n in heads], 1))
        wk = np.ascontiguousarray(np.concatenate([W_K[n] for n in heads], 1))
        wv = np.ascontiguousarray(np.concatenate([W_V[n] for n in heads], 1))
        # z rows within pair g: [0:64] = head 2g+1, [64:128] = head 2g
        wo = np.ascontiguousarray(np.concatenate(
            [W_O[heads[1]], W_O[heads[0]], W_O[heads[3]], W_O[heads[2]]], 0))
        import ml_dtypes
        bf = ml_dtypes.bfloat16
        in_maps.append({"xT": xT.astype(bf), "wq": wq.astype(bf),
                        "wk": wk.astype(bf), "wv": wv.astype(bf),
                        "wo": wo.astype(bf), "mask": mask})
    return in_maps


def run(in_maps, **kw):
    from concourse.bass_utils import run_bass_kernel_spmd
    return run_bass_kernel_spmd(get_program(), in_maps,
                                core_ids=list(range(8)), **kw)


def kernel(normalized_resid_pre, W_Q, W_K, W_V, W_O):
    in_maps = make_in_maps(normalized_resid_pre, W_Q, W_K, W_V, W_O)
    res = run(in_maps)
    out = np.empty((4, S, D), np.float32)
    for b in range(4):
        out[b] = res.results[2 * b]["o"] + res.results[2 * b + 1]["o"]
    return out



# revision 10
# speedup vs baseline: 1.1794x; 1.1794x over previous
"""Grouped multi-query attention (group axis summed) on 8 trn2 NeuronCores.

Math: reference sums the g axis of the grouped Q heads inside the score
einsum, so the whole module collapses to standard 8-head attention with
W_Qeff[n] = sum_g W_Q[4n+g] (and the 1/sqrt(64) score scale folded in).

Sharding: core c -> (batch b = c//2, kv-head half = c%2). Each core runs
4 heads of one batch and produces a full [2048, 2048] partial of the
output projection; the host sums the two halves per batch.

Per-core layout (all matmuls are out = lhsT.T @ rhs, bf16 in / f32 acc):
  xT [d, t] host-pretransposed; Q_T/K_T [2*64 head-pair rows, t] built by
  projection matmuls; V built directly with a ones column appended per
  head ([128, nkt, 4 heads, 64+1]) so each PV matmul also accumulates the
  softmax denominator into psum row 64 for free; scores computed
  transposed S_T[k, q] = K @ Q^T so softmax/PV need no transposes.
  Causal structure: for diagonal-group key blocks (m = ik - 4*jq >= 0)
  only columns [128m, 512) of the q-block are computed (score matmul,
  exp, PV all shrunk); the triangular [128,128] sub-block gets a
  multiplicative bf16 mask on DVE (4x mode). Per (jq, g, head): Pool
  copies z+den psum rows to SBUF bf16 (frees the bank fast), DVE takes
  the reciprocal of the den row, Pool partition-broadcasts it, DVE does
  the normalize multiply in 4x mode. Out-projection is interleaved per
  q-block round and DMA'd to HBM directly from PSUM.
"""

import numpy as np

S = 2048
D = 2048
HD = 64
NKV = 8
GQ = 4  # grouped q heads per kv head (summed)
HPC = 4  # heads per core
TB = 512
QB = 512
NTB = S // TB
NDC = D // 128
NQB = S // QB
NKT = S // 128
IGNORE = -100000.0

_CACHE = {}


def _build_program():
    import concourse.bass as bass
    import concourse.tile as tile
    from concourse import bacc, mybir

    F32 = mybir.dt.float32
    BF16 = mybir.dt.bfloat16
    AF = mybir.ActivationFunctionType

    nc = bacc.Bacc("TRN2", target_bir_lowering=False, debug=False,
                   enable_asserts=False, num_devices=8)

    xT_d = nc.dram_tensor("xT", [D, S], BF16, kind="ExternalInput").ap()
    wq_d = nc.dram_tensor("wq", [D, HPC * HD], BF16, kind="ExternalInput").ap()
    wk_d = nc.dram_tensor("wk", [D, HPC * HD], BF16, kind="ExternalInput").ap()
    wv_d = nc.dram_tensor("wv", [D, HPC * HD], BF16, kind="ExternalInput").ap()
    wo_d = nc.dram_tensor("wo", [HPC * HD, D], BF16, kind="ExternalInput").ap()
    mask_d = nc.dram_tensor("mask", [128, 128], BF16, kind="ExternalInput").ap()
    o_d = nc.dram_tensor("o", [S, D], BF16, kind="ExternalOutput").ap()

    with tile.TileContext(nc) as tc, \
            nc.allow_low_precision(reason="bf16 matmul operands by design"):
        with (
            tc.tile_pool(name="singles", bufs=1) as singles,
            tc.tile_pool(name="persist", bufs=1) as persist,
            tc.tile_pool(name="work", bufs=4) as work,
            tc.tile_pool(name="tiny", bufs=4) as tiny,
            tc.tile_pool(name="outsb", bufs=3) as outsb,
        ):
            # constants + weights
            wq_sb = singles.tile([128, NDC, HPC * HD], BF16, tag="wq")
            wk_sb = singles.tile([128, NDC, HPC * HD], BF16, tag="wk")
            wv_sb = singles.tile([128, NDC, HPC * HD], BF16, tag="wv")
            wo_sb = singles.tile([128, 2, D], BF16, tag="wo")
            mask_sb = singles.tile([128, 128], BF16, tag="mask")
            ones_row = singles.tile([1, HD], BF16, tag="oner")
            nc.vector.memset(ones_row, 1.0)

            nc.sync.dma_start(out=wq_sb, in_=wq_d.rearrange("(c p) n -> p c n", p=128))
            nc.sync.dma_start(out=wk_sb, in_=wk_d.rearrange("(c p) n -> p c n", p=128))
            nc.sync.dma_start(out=wv_sb, in_=wv_d.rearrange("(c p) n -> p c n", p=128))
            nc.sync.dma_start(out=wo_sb, in_=wo_d.rearrange("(g p) d -> p g d", p=128))
            nc.sync.dma_start(out=mask_sb, in_=mask_d)

            qT_sb = persist.tile([128, 2, S], BF16, tag="qT")
            kT_sb = persist.tile([128, 2, S], BF16, tag="kT")
            # per head: 64 V columns + a ones column (softmax denominator)
            v_sb = persist.tile([128, NKT, HPC, HD + 1], BF16, tag="v")
            z_sb = persist.tile([128, 2, S], BF16, tag="z")
            nc.vector.memset(v_sb[:, :, :, HD:HD + 1], 1.0)

            # ---- phase 1: projections ----
            with tc.tile_pool(name="ph1ps", bufs=4,
                              space=bass.MemorySpace.PSUM) as ph1ps:
                for tb in range(NTB):
                    ps_q = [ph1ps.tile([128, TB], F32, tag="qk", name=f"psq{tb}_{i}") for i in range(2)]
                    ps_k = [ph1ps.tile([128, TB], F32, tag="qk", name=f"psk{tb}_{i}") for i in range(2)]
                    ps_v = [ph1ps.tile([128, HPC * HD], F32, tag="v", name=f"psv{tb}_{i}") for i in range(4)]
                    for dc in range(NDC):
                        xt = work.tile([128, TB], BF16, tag="xt")
                        nc.sync.dma_start(
                            out=xt,
                            in_=xT_d[dc * 128:(dc + 1) * 128, tb * TB:(tb + 1) * TB])
                        st = dict(start=(dc == 0), stop=(dc == NDC - 1))
                        for g in range(2):
                            nc.tensor.matmul(
                                ps_q[g], wq_sb[:, dc, 128 * g:128 * (g + 1)],
                                xt, **st)
                            nc.tensor.matmul(
                                ps_k[g], wk_sb[:, dc, 128 * g:128 * (g + 1)],
                                xt, **st)
                        for tt in range(4):
                            nc.tensor.matmul(
                                ps_v[tt], xt[:, tt * 128:(tt + 1) * 128],
                                wv_sb[:, dc, :], **st)
                    for g in range(2):
                        nc.scalar.copy(out=qT_sb[:, g, tb * TB:(tb + 1) * TB], in_=ps_q[g])
                        nc.scalar.copy(out=kT_sb[:, g, tb * TB:(tb + 1) * TB], in_=ps_k[g])
                    for tt in range(4):
                        nc.vector.tensor_copy(
                            out=v_sb[:, 4 * tb + tt, :, 0:HD],
                            in_=ps_v[tt].rearrange("p (h c) -> p h c", h=HPC))

            # ---- phase 2: attention, phase 3: output projection ----
            with (
                tc.tile_pool(name="sps", bufs=4, space=bass.MemorySpace.PSUM) as sps,
                tc.tile_pool(name="zps", bufs=2, space=bass.MemorySpace.PSUM) as zps,
                tc.tile_pool(name="ops", bufs=2, space=bass.MemorySpace.PSUM) as ops,
            ):
                for jq in range(NQB):
                    nkt = 4 * (jq + 1)
                    for g in range(2):
                        ps_z = [zps.tile([128, QB], F32, tag="z",
                                         name=f"z{jq}_{g}_{jj}") for jj in range(2)]
                        for ik in range(nkt):
                            m = ik - 4 * jq
                            co = 128 * m if m > 0 else 0
                            p2 = []
                            for jj in range(2):
                                ps_s = sps.tile([128, QB], F32, tag="s",
                                                name=f"s{jq}_{g}_{ik}_{jj}")
                                nc.tensor.matmul(
                                    ps_s[:, co:QB],
                                    kT_sb[64 * jj:64 * (jj + 1), g,
                                          ik * 128:(ik + 1) * 128],
                                    qT_sb[64 * jj:64 * (jj + 1), g,
                                          jq * QB + co:(jq + 1) * QB],
                                    start=True, stop=True)
                                p_sb = work.tile([128, QB], BF16, tag="p",
                                                 name=f"p{jq}_{g}_{ik}_{jj}")
                                nc.scalar.activation(out=p_sb[:, co:QB],
                                                     in_=ps_s[:, co:QB],
                                                     func=AF.Exp)
                                if m >= 0:
                                    nc.vector.tensor_mul(
                                        p_sb[:, co:co + 128],
                                        p_sb[:, co:co + 128], mask_sb)
                                p2.append(p_sb)
                            st = dict(start=(ik == 0), stop=(ik == nkt - 1),
                                      skip_group_check=True)
                            for jj, p_sb in enumerate(p2):
                                nc.tensor.matmul(
                                    ps_z[jj][0:HD + 1, co:QB],
                                    v_sb[:, ik, 2 * g + jj, :],
                                    p_sb[:, co:QB], **st)
                        # stage z+den to SBUF (frees the psum banks for the
                        # broadcast matmuls), then normalize
                        zraws = []
                        for jj in range(2):
                            zraw = tiny.tile([HD + 1, QB], BF16, tag="zraw",
                                             name=f"zr{jq}_{g}_{jj}")
                            nc.vector.tensor_copy(out=zraw,
                                                  in_=ps_z[jj][0:HD + 1, :])
                            zraws.append(zraw)
                        for jj, zraw in enumerate(zraws):
                            rj = 1 - jj
                            rec = tiny.tile([1, QB], BF16, tag="rec",
                                            name=f"re{jq}_{g}_{jj}")
                            nc.vector.reciprocal(out=rec,
                                                 in_=zraw[HD:HD + 1, :])
                            ps_bc = zps.tile([128, QB], F32, tag="z",
                                             name=f"bc{jq}_{g}_{jj}")
                            nc.tensor.matmul(ps_bc[0:HD, :], ones_row, rec,
                                             start=True, stop=True)
                            nc.vector.tensor_mul(
                                z_sb[64 * rj:64 * (rj + 1), g,
                                     jq * QB:(jq + 1) * QB],
                                zraw[0:HD, :], ps_bc[0:HD, :])

                    for it in range(4 * jq, 4 * jq + 4):
                        for db in range(4):
                            ps_o = ops.tile([128, 512], F32, tag="o",
                                            name=f"o{it}_{db}")
                            for g in range(2):
                                nc.tensor.matmul(
                                    ps_o, z_sb[:, g, it * 128:(it + 1) * 128],
                                    wo_sb[:, g, db * 512:(db + 1) * 512],
                                    start=(g == 0), stop=(g == 1))
                            o_sb = outsb.tile([128, 512], BF16, tag="o",
                                              name=f"os{it}_{db}")
                            nc.vector.tensor_copy(out=o_sb, in_=ps_o)
                            nc.sync.dma_start(
                                out=o_d[it * 128:(it + 1) * 128,
                                        db * 512:(db + 1) * 512],
                                in_=o_sb)

    nc.compile()
    return nc


def get_program():
    if "nc" not in _CACHE:
        _CACHE["nc"] = _build_program()
    return _CACHE["nc"]


def make_in_maps(normalized_resid_pre, W_Q, W_K, W_V, W_O):
    x = normalized_resid_pre
    x = np.ascontiguousarray(np.asarray(x, np.float32))
    W_Q = np.asarray(W_Q, np.float32)
    W_K = np.asarray(W_K, np.float32)
    W_V = np.asarray(W_V, np.float32)
    W_O = np.asarray(W_O, np.float32)
    wqe = W_Q.reshape(NKV, GQ, D, HD).sum(1) * (1.0 / np.sqrt(HD))

    import ml_dtypes
    bf = ml_dtypes.bfloat16
    kk = np.arange(128)[:, None]
    qq = np.arange(128)[None, :]
    mask = np.ascontiguousarray(np.where(kk <= qq, 1.0, 0.0).astype(bf))

    in_maps = []
    for c in range(8):
        b, half = divmod(c, 2)
        heads = [4 * half + m for m in range(HPC)]
        xT = np.ascontiguousarray(x[b].T)
        wq = np.ascontiguousarray(np.concatenate([wqe[n] for n in heads], 1))
        wk = np.ascontiguousarray(np.concatenate([W_K[n] for n in heads], 1))
        wv = np.ascontiguousarray(np.concatenate([W_V[n] for n in heads], 1))
        # z rows within pair g: [0:64] = head 2g+1, [64:128] = head 2g
        wo = np.ascontiguousarray(np.concatenate(
            [W_O[heads[1]], W_O[heads[0]], W_O[heads[3]], W_O[heads[2]]], 0))
        in_maps.append({"xT": xT.astype(bf), "wq": wq.astype(bf),
                        "wk": wk.astype(bf), "wv": wv.astype(bf),
                        "wo": wo.astype(bf), "mask": mask})
    return in_maps


def run(in_maps, **kw):
    from concourse.bass_utils import run_bass_kernel_spmd
    return run_bass_kernel_spmd(get_program(), in_maps,
                                core_ids=list(range(8)), **kw)


def kernel(normalized_resid_pre, W_Q, W_K, W_V, W_O):
    in_maps = make_in_maps(normalized_resid_pre, W_Q, W_K, W_V, W_O)
    res = run(in_maps)
    out = np.empty((4, S, D), np.float32)
    for b in range(4):
        out[b] = (res.results[2 * b]["o"].astype(np.float32)
                  + res.results[2 * b + 1]["o"].astype(np.float32))
    return out


# revision 11
# speedup vs baseline: 1.2719x; 1.0784x over previous
"""Grouped multi-query attention (group axis summed) on 8 trn2 NeuronCores.

Math: reference sums the g axis of the grouped Q heads inside the score
einsum, so the whole module collapses to standard 8-head attention with
W_Qeff[n] = sum_g W_Q[4n+g] (and the 1/sqrt(64) score scale folded in).

Sharding: core c -> (batch b = c//2, kv-head half = c%2). Each core runs
4 heads of one batch and produces a full [2048, 2048] partial of the
output projection (bf16); the host sums the two halves per batch.

Per-core layout (all matmuls are out = lhsT.T @ rhs, bf16 in / f32 acc):
  xT [d, t] host-pretransposed; Q_T/K_T [2*64 head-pair rows, t] built by
  projection matmuls (weights DMA'd in per-dc-pair chunks interleaved
  ahead of use); V built token-major with a ones column appended per head
  ([128, nkt, 4 heads, 64+1]) so each PV matmul also accumulates the
  softmax denominator into psum row 64 for free; scores computed
  transposed S_T[k, q] = K @ Q^T so softmax/PV need no transposes.
  Key blocks are processed in pairs sharing a [128, 1024] two-bank psum
  tile so off-diagonal pairs need a single merged exp. Causal structure:
  diagonal-group key blocks (m = ik - 4*jq >= 0) only compute columns
  [128m, 512) (score matmul, exp, PV all shrunk); the triangular
  [128,128] sub-block gets a multiplicative bf16 mask on DVE (4x mode).
  Normalization per (jq, g, head): DVE reciprocal straight off the psum
  den row, K=1 ones-matmul broadcasts it over 64 psum partitions (ops
  pool), DVE multiplies the SBUF-staged z rows. The output projection of
  q-block jq-1 is interleaved one [128,512] d-tile at a time into jq's
  key-block loop (PE fills Act-bound bubbles; DVE o-copies spread out).
"""

import numpy as np

S = 2048
D = 2048
HD = 64
NKV = 8
GQ = 4  # grouped q heads per kv head (summed)
HPC = 4  # heads per core
TB = 512
QB = 512
NTB = S // TB
NDC = D // 128
NQB = S // QB
NKT = S // 128
IGNORE = -100000.0

_CACHE = {}


def _build_program():
    import concourse.bass as bass
    import concourse.tile as tile
    from concourse import bacc, mybir

    F32 = mybir.dt.float32
    BF16 = mybir.dt.bfloat16
    AF = mybir.ActivationFunctionType

    nc = bacc.Bacc("TRN2", target_bir_lowering=False, debug=False,
                   enable_asserts=False, num_devices=8)

    xT_d = nc.dram_tensor("xT", [D, S], BF16, kind="ExternalInput").ap()
    wq_d = nc.dram_tensor("wq", [D, HPC * HD], BF16, kind="ExternalInput").ap()
    wk_d = nc.dram_tensor("wk", [D, HPC * HD], BF16, kind="ExternalInput").ap()
    wv_d = nc.dram_tensor("wv", [D, HPC * HD], BF16, kind="ExternalInput").ap()
    wo_d = nc.dram_tensor("wo", [HPC * HD, D], BF16, kind="ExternalInput").ap()
    mask_d = nc.dram_tensor("mask", [128, 128], BF16, kind="ExternalInput").ap()
    o_d = nc.dram_tensor("o", [S, D], BF16, kind="ExternalOutput").ap()

    with tile.TileContext(nc) as tc, \
            nc.allow_low_precision(reason="bf16 matmul operands by design"):
        with (
            tc.tile_pool(name="singles", bufs=1) as singles,
            tc.tile_pool(name="persist", bufs=1) as persist,
            tc.tile_pool(name="work", bufs=4) as work,
            tc.tile_pool(name="tiny", bufs=4) as tiny,
            tc.tile_pool(name="outsb", bufs=3) as outsb,
        ):
            wq_sb = singles.tile([128, NDC, HPC * HD], BF16, tag="wq")
            wk_sb = singles.tile([128, NDC, HPC * HD], BF16, tag="wk")
            wv_sb = singles.tile([128, NDC, HPC * HD], BF16, tag="wv")
            wo_sb = singles.tile([128, 2, D], BF16, tag="wo")
            mask_sb = singles.tile([128, 128], BF16, tag="mask")
            ones_row = singles.tile([1, HD], BF16, tag="oner")
            nc.vector.memset(ones_row, 1.0)

            def w_chunk(sb, d, k):  # dc pair k -> dcs 2k, 2k+1
                nc.sync.dma_start(
                    out=sb[:, 2 * k:2 * k + 2, :],
                    in_=d[2 * k * 128:(2 * k + 2) * 128, :]
                    .rearrange("(c p) n -> p c n", p=128))

            for k in range(2):  # first two dc-pairs of each weight up front
                w_chunk(wq_sb, wq_d, k)
                w_chunk(wk_sb, wk_d, k)
                w_chunk(wv_sb, wv_d, k)
            nc.sync.dma_start(out=mask_sb, in_=mask_d)

            qT_sb = persist.tile([128, 2, S], BF16, tag="qT")
            kT_sb = persist.tile([128, 2, S], BF16, tag="kT")
            # per head: 64 V columns + a ones column (softmax denominator)
            v_sb = persist.tile([128, NKT, HPC, HD + 1], BF16, tag="v")
            z_sb = persist.tile([128, 2, S], BF16, tag="z")
            nc.vector.memset(v_sb[:, :, :, HD:HD + 1], 1.0)

            # ---- phase 1: projections ----
            with tc.tile_pool(name="ph1ps", bufs=4,
                              space=bass.MemorySpace.PSUM) as ph1ps:
                for tb in range(NTB):
                    ps_q = [ph1ps.tile([128, TB], F32, tag="qk", name=f"psq{tb}_{i}") for i in range(2)]
                    ps_k = [ph1ps.tile([128, TB], F32, tag="qk", name=f"psk{tb}_{i}") for i in range(2)]
                    ps_v = [ph1ps.tile([128, HPC * HD], F32, tag="v", name=f"psv{tb}_{i}") for i in range(4)]
                    for dc in range(NDC):
                        xt = work.tile([128, TB], BF16, tag="xt")
                        nc.sync.dma_start(
                            out=xt,
                            in_=xT_d[dc * 128:(dc + 1) * 128, tb * TB:(tb + 1) * TB])
                        if tb == 0 and dc < NDC // 2 - 2:
                            w_chunk(wq_sb, wq_d, dc + 2)
                            w_chunk(wk_sb, wk_d, dc + 2)
                            w_chunk(wv_sb, wv_d, dc + 2)
                        if tb == 1 and dc == 0:
                            nc.sync.dma_start(
                                out=wo_sb,
                                in_=wo_d.rearrange("(g p) d -> p g d", p=128))
                        st = dict(start=(dc == 0), stop=(dc == NDC - 1))
                        for g in range(2):
                            nc.tensor.matmul(
                                ps_q[g], wq_sb[:, dc, 128 * g:128 * (g + 1)],
                                xt, **st)
                            nc.tensor.matmul(
                                ps_k[g], wk_sb[:, dc, 128 * g:128 * (g + 1)],
                                xt, **st)
                        for tt in range(4):
                            nc.tensor.matmul(
                                ps_v[tt], xt[:, tt * 128:(tt + 1) * 128],
                                wv_sb[:, dc, :], **st)
                    for g in range(2):
                        nc.scalar.copy(out=qT_sb[:, g, tb * TB:(tb + 1) * TB], in_=ps_q[g])
                        nc.scalar.copy(out=kT_sb[:, g, tb * TB:(tb + 1) * TB], in_=ps_k[g])
                    for tt in range(4):
                        nc.vector.tensor_copy(
                            out=v_sb[:, 4 * tb + tt, :, 0:HD],
                            in_=ps_v[tt].rearrange("p (h c) -> p h c", h=HPC))

            # ---- phase 2: attention, + interleaved output projection ----
            with (
                tc.tile_pool(name="sps", bufs=2, space=bass.MemorySpace.PSUM) as sps,
                tc.tile_pool(name="zps", bufs=2, space=bass.MemorySpace.PSUM) as zps,
                tc.tile_pool(name="ops", bufs=2, space=bass.MemorySpace.PSUM) as ops,
            ):
                pending = []  # (it, db) out-proj tiles of the previous jq

                def emit_oproj(n):
                    for _ in range(min(n, len(pending))):
                        it, db = pending.pop(0)
                        ps_o = ops.tile([128, 512], F32, tag="o",
                                        name=f"o{it}_{db}")
                        for g in range(2):
                            nc.tensor.matmul(
                                ps_o, z_sb[:, g, it * 128:(it + 1) * 128],
                                wo_sb[:, g, db * 512:(db + 1) * 512],
                                start=(g == 0), stop=(g == 1))
                        o_sb = outsb.tile([128, 512], BF16, tag="o",
                                          name=f"os{it}_{db}")
                        nc.vector.tensor_copy(out=o_sb, in_=ps_o)
                        nc.sync.dma_start(
                            out=o_d[it * 128:(it + 1) * 128,
                                    db * 512:(db + 1) * 512],
                            in_=o_sb)

                for jq in range(NQB):
                    nkt = 4 * (jq + 1)
                    npair = nkt // 2
                    iters = 2 * npair
                    per_iter = -(-len(pending) // iters) if pending else 0
                    for g in range(2):
                        ps_z = [zps.tile([128, QB], F32, tag="z",
                                         name=f"z{jq}_{g}_{jj}") for jj in range(2)]
                        for u in range(npair):
                            iks = (2 * u, 2 * u + 1)
                            ms = [ik - 4 * jq for ik in iks]
                            cos = [128 * m if m > 0 else 0 for m in ms]
                            p2 = []
                            for jj in range(2):
                                ps_s = sps.tile([128, 2 * QB], F32, tag="s",
                                                name=f"s{jq}_{g}_{u}_{jj}")
                                p_sb = work.tile([128, 2 * QB], BF16, tag="p",
                                                 name=f"p{jq}_{g}_{u}_{jj}")
                                for w, ik in enumerate(iks):
                                    co = cos[w]
                                    nc.tensor.matmul(
                                        ps_s[:, QB * w + co:QB * (w + 1)],
                                        kT_sb[64 * jj:64 * (jj + 1), g,
                                              ik * 128:(ik + 1) * 128],
                                        qT_sb[64 * jj:64 * (jj + 1), g,
                                              jq * QB + co:(jq + 1) * QB],
                                        start=True, stop=True)
                                if ms[0] >= 0:  # diagonal pair: shrunk exps
                                    for w in range(2):
                                        co = cos[w]
                                        nc.scalar.activation(
                                            out=p_sb[:, QB * w + co:QB * (w + 1)],
                                            in_=ps_s[:, QB * w + co:QB * (w + 1)],
                                            func=AF.Exp)
                                else:  # merged exp over both key blocks
                                    nc.scalar.activation(out=p_sb, in_=ps_s,
                                                         func=AF.Exp)
                                for w in range(2):
                                    if ms[w] >= 0:
                                        co = cos[w]
                                        nc.vector.tensor_mul(
                                            p_sb[:, QB * w + co:QB * w + co + 128],
                                            p_sb[:, QB * w + co:QB * w + co + 128],
                                            mask_sb)
                                p2.append(p_sb)
                            for jj, p_sb in enumerate(p2):
                                for w, ik in enumerate(iks):
                                    co = cos[w]
                                    nc.tensor.matmul(
                                        ps_z[jj][0:HD + 1, co:QB],
                                        v_sb[:, ik, 2 * g + jj, :],
                                        p_sb[:, QB * w + co:QB * (w + 1)],
                                        start=(ik == 0), stop=(ik == nkt - 1),
                                        skip_group_check=True)
                            emit_oproj(per_iter)
                        # normalize: recip straight off the psum den row, then
                        # stage z to SBUF, ones-matmul broadcast, multiply
                        recs = []
                        for jj in range(2):
                            rec = tiny.tile([1, QB], BF16, tag="rec",
                                            name=f"re{jq}_{g}_{jj}")
                            nc.vector.reciprocal(out=rec,
                                                 in_=ps_z[jj][HD:HD + 1, :])
                            recs.append(rec)
                        zraws = []
                        for jj in range(2):
                            zraw = tiny.tile([HD, QB], BF16, tag="zraw",
                                             name=f"zr{jq}_{g}_{jj}")
                            nc.vector.tensor_copy(out=zraw,
                                                  in_=ps_z[jj][0:HD, :])
                            zraws.append(zraw)
                        for jj in range(2):
                            rj = 1 - jj
                            ps_bc = ops.tile([128, QB], F32, tag="o",
                                             name=f"bc{jq}_{g}_{jj}")
                            nc.tensor.matmul(ps_bc[0:HD, :], ones_row,
                                             recs[jj], start=True, stop=True)
                            nc.vector.tensor_mul(
                                z_sb[64 * rj:64 * (rj + 1), g,
                                     jq * QB:(jq + 1) * QB],
                                zraws[jj], ps_bc[0:HD, :])
                    emit_oproj(len(pending))
                    pending = [(it, db) for it in range(4 * jq, 4 * jq + 4)
                               for db in range(4)]
                emit_oproj(len(pending))

    nc.compile()
    return nc


def get_program():
    if "nc" not in _CACHE:
        _CACHE["nc"] = _build_program()
    return _CACHE["nc"]


def make_in_maps(normalized_resid_pre, W_Q, W_K, W_V, W_O):
    x = normalized_resid_pre
    x = np.ascontiguousarray(np.asarray(x, np.float32))
    W_Q = np.asarray(W_Q, np.float32)
    W_K = np.asarray(W_K, np.float32)
    W_V = np.asarray(W_V, np.float32)
    W_O = np.asarray(W_O, np.float32)
    wqe = W_Q.reshape(NKV, GQ, D, HD).sum(1) * (1.0 / np.sqrt(HD))

    import ml_dtypes
    bf = ml_dtypes.bfloat16
    kk = np.arange(128)[:, None]
    qq = np.arange(128)[None, :]
    mask = np.ascontiguousarray(np.where(kk <= qq, 1.0, 0.0).astype(bf))

    in_maps = []
    for c in range(8):
        b, half = divmod(c, 2)
        heads = [4 * half + m for m in range(HPC)]
        xT = np.ascontiguousarray(x[b].T)
        wq = np.ascontiguousarray(np.concatenate([wqe[n] for n in heads], 1))
        wk = np.ascontiguousarray(np.concatenate([W_K[n] for n in heads], 1))
        wv = np.ascontiguousarray(np.concatenate([W_V[n] for n in heads], 1))
        # z rows within pair g: [0:64] = head 2g+1, [64:128] = head 2g
        wo = np.ascontiguousarray(np.concatenate(
            [W_O[heads[1]], W_O[heads[0]], W_O[heads[3]], W_O[heads[2]]], 0))
        in_maps.append({"xT": xT.astype(bf), "wq": wq.astype(bf),
                        "wk": wk.astype(bf), "wv": wv.astype(bf),
                        "wo": wo.astype(bf), "mask": mask})
    return in_maps


def run(in_maps, **kw):
    from concourse.bass_utils import run_bass_kernel_spmd
    return run_bass_kernel_spmd(get_program(), in_maps,
                                core_ids=list(range(8)), **kw)


def kernel(normalized_resid_pre, W_Q, W_K, W_V, W_O):
    in_maps = make_in_maps(normalized_resid_pre, W_Q, W_K, W_V, W_O)
    res = run(in_maps)
    out = np.empty((4, S, D), np.float32)
    for b in range(4):
        out[b] = (res.results[2 * b]["o"].astype(np.float32)
                  + res.results[2 * b + 1]["o"].astype(np.float32))
    return out
